# revision 6
# baseline (speedup 1.0000x reference)
"""Trainium2 Bass kernel for nn_ChannelAttentionModule.

Per batch element b (one NeuronCore each, pure data parallel over B=8):
    f = x[b].reshape(C, N)                      # C=64, N=4096
    A = f^T f                                   # (N, N) symmetric
    P = softmax(A, axis=-1)                     # row softmax
    out = x + (f @ P).reshape(C, H, W)

Streaming formulation (never materializes A in HBM): for each row-tile m
(128 rows), compute A[m, :] via matmul, E = exp(A[m, :] - D[m]) where
D[m] = A[m, m] = ||f_m||^2 (a valid softmax shift: row max <= max_n ||f_n||^2
by Cauchy-Schwarz and A[m,m] is in the row, so exponents stay in [-inf, ~21]),
accumulate Z[m] = sum_n E[m, n] via the activation's accum_out, then
out += (f_m / Z[m]) @ E via PSUM-accumulated matmuls.

Output chunks are partition-packed in PSUM (odd chunks at partitions 64-127
via tensor-engine column tiling) so the [64, 4096] accumulator fits in 4
banks, leaving 4 banks for double-buffered A tiles.
"""

import numpy as np

import concourse.bass as bass
from concourse import mybir
from concourse.bass_utils import run_bass_kernel_spmd
from concourse.masks import make_identity
from concourse.tile import TileContext

B, C, H, W = 8, 64, 64, 64
N = H * W              # 4096
P = 128                # rows per m-tile
NT = N // P            # 32 m-tiles
MM = 512               # matmul moving-operand width (fp32 max / one PSUM bank)
ACH = 1024             # A-chunk width seen by one exp activation (2 banks)
NACH = N // ACH        # 4 exp chunks per m-tile
F32 = mybir.dt.float32
BF16 = mybir.dt.bfloat16

_MAX_WAITS = 1


def _split_waits(nc, max_waits=_MAX_WAITS):
    """The walrus build in this container rejects instructions carrying more
    than a couple of semaphore waits ("Too many sync wait commands").  Hoist
    extra waits onto InstNoOp instructions inserted just before, on the same
    engine (engine executes them in order, so semantics are identical)."""
    for fn in nc.m.functions:
        for bb in fn.blocks:
            new_insts = []
            for inst in bb.instructions:
                si = inst.sync_info
                if si is not None and si.on_wait and len(si.on_wait) > max_waits:
                    waits = list(si.on_wait)
                    for j, wcond in enumerate(waits[max_waits:]):
                        new_insts.append(
                            mybir.InstNoOp(
                                name=f"{inst.name}-ws{j}",
                                engine=inst.engine,
                                ins=[],
                                outs=[],
                                sync_info=mybir.SyncInfo(
                                    on_wait=[wcond], on_update=[]
                                ),
                            )
                        )
                    si.on_wait = waits[:max_waits]
                new_insts.append(inst)
            bb.instructions[:] = new_insts
    return nc


def build(mm_dt_name="float32r", repeats=1):
    """Build the per-core Bass module.  mm_dt_name picks the matmul operand
    dtype: 'float32r' (full PE rate, reduced precision) or 'float32'
    (4x slower, exact).  repeats>1 re-runs the whole body for timing.

    The BIR verifier requires every operand of an fp32r matmul to be
    *produced* with dtype float32r, so the matmul-feeding tiles (f2, e_t,
    sfT) are declared float32r; everything else keeps fp32 views of the
    same bytes."""
    mm_dt = getattr(mybir.dt, mm_dt_name)
    is_r = mm_dt != F32

    nc = bass.Bass()
    x = nc.dram_tensor("x", [C, N], F32, kind="ExternalInput")
    y = nc.dram_tensor("y", [C, N], F32, kind="ExternalOutput")

    with TileContext(nc) as tc:
        with (
            tc.tile_pool(name="big", bufs=1) as big,
            tc.tile_pool(name="erow", bufs=2) as erow,
            tc.tile_pool(name="small", bufs=4) as small,
            tc.tile_pool(name="opsum", bufs=1, space="PSUM") as opsum,
            tc.tile_pool(name="apsum", bufs=2, space="PSUM") as apsum,
        ):
            for _ in range(repeats):
                # ---- load f into both partition halves --------------------
                f2 = big.tile([P, N], mm_dt, tag="f2")
                xin = x[:, :].bitcast(mm_dt) if is_r else x[:, :]
                nc.sync.dma_start(out=f2[0:C, :], in_=xin)
                nc.sync.dma_start(out=f2[C:P, :], in_=xin)
                f2f = f2.bitcast(F32) if is_r else f2  # exact-fp32 view

                ident = big.tile([C, C], F32, tag="ident")
                make_identity(nc, ident)

                # ---- fT tiles: fT[p, i*C + c] = f[c, i*P + p] -------------
                fT = big.tile([P, NT * C], F32, tag="fT")
                for g in range(2):
                    tp = apsum.tile([P, ACH], F32, tag="a_t")
                    for k in range(16):
                        i = g * 16 + k
                        nc.tensor.transpose(
                            tp[:, k * C:(k + 1) * C],
                            f2f[0:C, i * P:(i + 1) * P],
                            ident,
                        )
                    nc.vector.tensor_copy(
                        fT[:, g * 16 * C:(g + 1) * 16 * C], tp
                    )

                # ---- negD[p, i] = -||f[:, i*P+p]||^2 ----------------------
                fsq = big.tile([P, NT * C], F32, tag="fsq")
                nc.vector.tensor_mul(fsq, fT, fT)
                negD = big.tile([P, NT], F32, tag="negD")
                nc.vector.tensor_reduce(
                    negD,
                    fsq.rearrange("p (t c) -> p t c", c=C),
                    axis=mybir.AxisListType.X,
                    op=mybir.AluOpType.add,
                    negate=True,
                )

                # ---- main loop over row tiles -----------------------------
                o_t = opsum.tile([P, 4 * MM], F32, tag="o_t")  # 4 banks
                for i in range(NT):
                    e_t = erow.tile([P, N], BF16, tag="e_t")
                    zparts = small.tile([P, NACH], F32, tag="zparts")
                    lhs1 = f2[0:C, i * P:(i + 1) * P]
                    for a in range(NACH):
                        a_t = apsum.tile([P, ACH], F32, tag="a_t")
                        for h in range(2):
                            col = a * ACH + h * MM
                            nc.tensor.matmul(
                                a_t[:, h * MM:(h + 1) * MM],
                                lhs1,
                                f2[0:C, col:col + MM],
                                start=True,
                                stop=True,
                                skip_group_check=True,
                            )
                        nc.scalar.activation(
                            e_t[:, a * ACH:(a + 1) * ACH],
                            a_t,
                            mybir.ActivationFunctionType.Exp,
                            bias=negD[:, i:i + 1],
                            scale=1.0,
                            accum_out=zparts[:, a:a + 1],
                        )
                    z = small.tile([P, 1], F32, tag="z")
                    nc.vector.tensor_reduce(
                        z, zparts, axis=mybir.AxisListType.X,
                        op=mybir.AluOpType.add,
                    )
                    zinv = small.tile([P, 1], F32, tag="zinv")
                    nc.vector.reciprocal(zinv, z)
                    sfT = small.tile([P, C], BF16, tag="sfT")
                    nc.vector.tensor_scalar_mul(
                        sfT, fT[:, i * C:(i + 1) * C], zinv
                    )
                    for j in range(8):
                        half, bank = j % 2, j // 2
                        nc.tensor.matmul(
                            o_t[half * C:(half + 1) * C,
                                bank * MM:(bank + 1) * MM],
                            sfT,
                            e_t[:, j * MM:(j + 1) * MM],
                            start=(i == 0),
                            stop=(i == NT - 1),
                            skip_group_check=True,
                        )

                # ---- final residual add + store ---------------------------
                # even chunks j=2k live at o_t[0:64, k*512:], odd at [64:128].
                out2 = big.tile([P, 4 * MM], F32, tag="out2")
                f2v = f2f.rearrange("p (k t m) -> p k t m", t=2, m=MM)
                for half in range(2):
                    nc.vector.tensor_add(
                        out2[half * C:(half + 1) * C, :].rearrange(
                            "p (k m) -> p k m", m=MM),
                        o_t[half * C:(half + 1) * C, :].rearrange(
                            "p (k m) -> p k m", m=MM),
                        f2v[half * C:(half + 1) * C, :, half, :],
                    )
                yv = y.rearrange("p (k t m) -> p k t m", t=2, m=MM)
                for half in range(2):
                    nc.sync.dma_start(
                        out=yv[:, :, half, :],
                        in_=out2[half * C:(half + 1) * C, :].rearrange(
                            "p (k m) -> p k m", m=MM),
                    )

    return nc


_NC_CACHE = {}


def _get_nc(mm_dt_name="float32r", repeats=1):
    key = (mm_dt_name, repeats)
    if key not in _NC_CACHE:
        _NC_CACHE[key] = _split_waits(build(mm_dt_name, repeats))
    return _NC_CACHE[key]


def run(x_full, mm_dt_name="float32r", repeats=1):
    """x_full: (B, C, H, W) fp32 -> (B, C, H, W) fp32, sharded over 8 cores."""
    x_full = np.ascontiguousarray(np.asarray(x_full, dtype=np.float32))
    assert x_full.shape == (B, C, H, W)
    nc = _get_nc(mm_dt_name, repeats)
    in_maps = [{"x": x_full[b].reshape(C, N)} for b in range(B)]
    res = run_bass_kernel_spmd(nc, in_maps, list(range(B)))
    out = np.stack([res.results[b]["y"] for b in range(B)])
    return out.reshape(B, C, H, W)


def kernel(**inputs):
    return run(inputs["x"])


# revision 13
# speedup vs baseline: 273.5214x; 273.5214x over previous
"""Trainium2 Bass kernel for nn_ChannelAttentionModule.

Per batch element b (one NeuronCore each, pure data parallel over B=8):
    f = x[b].reshape(C, N)                      # C=64, N=4096
    A = f^T f                                   # (N, N) symmetric
    P = softmax(A, axis=-1)                     # row softmax
    out = x + (f @ P).reshape(C, H, W)

Streaming formulation (never materializes A in HBM): for each row-tile m
(128 rows), compute A[m, :] via matmul, E = exp(A[m, :] - D[m]) where
D[m] = A[m, m] = ||f_m||^2 (a valid softmax shift: row max <= max_n ||f_n||^2
by Cauchy-Schwarz and A[m,m] is in the row, so exponents stay in [-inf, ~21]),
accumulate Z[m] = sum_n E[m, n] via the activation's accum_out, then
out += (f_m / Z[m]) @ E via PSUM-accumulated matmuls.

Output chunks are partition-packed in PSUM (odd chunks at partitions 64-127
via tensor-engine column tiling) so the [64, 4096] accumulator fits in 4
banks, leaving 4 banks for double-buffered A tiles.
"""

import numpy as np

import concourse.bass as bass
from concourse import mybir
from concourse.bass_utils import run_bass_kernel_spmd
from concourse.masks import make_identity
from concourse.tile import TileContext

B, C, H, W = 8, 64, 64, 64
N = H * W              # 4096
P = 128                # rows per m-tile
NT = N // P            # 32 m-tiles
MM = 512               # matmul moving-operand width (fp32 max / one PSUM bank)
ACH = 1024             # A-chunk width seen by one exp activation (2 banks)
NACH = N // ACH        # 4 exp chunks per m-tile
F32 = mybir.dt.float32
BF16 = mybir.dt.bfloat16

_MAX_WAITS = 1


def _split_waits(nc, max_waits=_MAX_WAITS):
    """The walrus build in this container rejects instructions carrying more
    than a couple of semaphore waits ("Too many sync wait commands").  Hoist
    extra waits onto InstNoOp instructions inserted just before, on the same
    engine (engine executes them in order, so semantics are identical)."""
    for fn in nc.m.functions:
        for bb in fn.blocks:
            new_insts = []
            for inst in bb.instructions:
                si = inst.sync_info
                if si is not None and si.on_wait and len(si.on_wait) > max_waits:
                    waits = list(si.on_wait)
                    for j, wcond in enumerate(waits[max_waits:]):
                        new_insts.append(
                            mybir.InstNoOp(
                                name=f"{inst.name}-ws{j}",
                                engine=inst.engine,
                                ins=[],
                                outs=[],
                                sync_info=mybir.SyncInfo(
                                    on_wait=[wcond], on_update=[]
                                ),
                            )
                        )
                    si.on_wait = waits[:max_waits]
                new_insts.append(inst)
            bb.instructions[:] = new_insts
    return nc


def build(mm_dt_name="float32r", repeats=1):
    """Build the per-core Bass module.  mm_dt_name picks the matmul operand
    dtype: 'float32r' (full PE rate, reduced precision) or 'float32'
    (4x slower, exact).  repeats>1 re-runs the whole body for timing.

    The BIR verifier requires every operand of an fp32r matmul to be
    *produced* with dtype float32r, so the matmul-feeding tiles (f2, e_t,
    sfT) are declared float32r; everything else keeps fp32 views of the
    same bytes."""
    mm_dt = getattr(mybir.dt, mm_dt_name)
    is_r = mm_dt != F32

    nc = bass.Bass()
    x = nc.dram_tensor("x", [C, N], F32, kind="ExternalInput")
    y = nc.dram_tensor("y", [C, N], F32, kind="ExternalOutput")

    with TileContext(nc) as tc:
        with (
            tc.tile_pool(name="big", bufs=1) as big,
            tc.tile_pool(name="erow", bufs=2) as erow,
            tc.tile_pool(name="small", bufs=4) as small,
            tc.tile_pool(name="opsum", bufs=1, space="PSUM") as opsum,
            tc.tile_pool(name="apsum", bufs=2, space="PSUM") as apsum,
        ):
            for _ in range(repeats):
                # ---- load f (chunked so compute starts early) -------------
                ident = big.tile([C, C], F32, tag="ident")
                make_identity(nc, ident)  # GPSIMD; issue before DMAs

                f2 = big.tile([P, N], mm_dt, tag="f2")
                xin = x[:, :].bitcast(mm_dt) if is_r else x[:, :]
                col = 0
                for w in (512, 512, 1024, 1024, 1024):
                    cs = slice(col, col + w)
                    nc.sync.dma_start(out=f2[0:C, cs], in_=xin[:, cs])
                    col += w
                nc.sync.dma_start(out=f2[C:P, :], in_=xin)
                f2f = f2.bitcast(F32) if is_r else f2  # exact-fp32 view

                # ---- fT tiles + negD, in 4 pipelined groups of 8 ----------
                # fT[p, i*C + c] = f[c, i*P + p];  negD[p, i] = -||f_m||^2.
                # Transposes stage through the o_t PSUM slot (unused until
                # the first mm2), keeping a_t free for mm1 from the start.
                fT = big.tile([P, NT * C], F32, tag="fT")
                fsq = big.tile([P, NT * C], F32, tag="fsq")
                negD = big.tile([P, NT], F32, tag="negD")
                tp = opsum.tile([P, 4 * MM], F32, tag="o_t")
                t0 = 0
                for ntile in (2, 6, 8, 8, 8):  # small first group: exp(0)
                    for i in range(t0, t0 + ntile):  # unblocks early
                        nc.tensor.transpose(
                            tp[:, i * C:(i + 1) * C],
                            f2f[0:C, i * P:(i + 1) * P],
                            ident,
                        )
                    gs = slice(t0 * C, (t0 + ntile) * C)
                    nc.vector.tensor_copy(fT[:, gs], tp[:, gs])
                    nc.vector.tensor_mul(fsq[:, gs], fT[:, gs], fT[:, gs])
                    nc.vector.tensor_reduce(
                        negD[:, t0:t0 + ntile],
                        fsq[:, gs].rearrange("p (t c) -> p t c", c=C),
                        axis=mybir.AxisListType.X,
                        op=mybir.AluOpType.add,
                        negate=True,
                    )
                    t0 += ntile

                # ---- main loop over row tiles -----------------------------
                # Software-pipelined: mm2 for iteration i-1 is emitted after
                # mm1+exp of iteration i, so the PE always has ready work
                # (mm2 can only start once iteration i-1's exps finished;
                # emitting it early would stall the PE queue and starve ACT).
                o_t = opsum.tile([P, 4 * MM], F32, tag="o_t")  # 4 banks
                out2 = big.tile([P, 4 * MM], F32, tag="out2")
                yv = y.rearrange("p (k t m) -> p k t m", t=2, m=MM)

                def emit_mm2(i, e_t, sfT):
                    last = i == NT - 1
                    for j in range(8):
                        half, bank = j % 2, j // 2
                        o_slice = o_t[half * C:(half + 1) * C,
                                      bank * MM:(bank + 1) * MM]
                        nc.tensor.matmul(
                            o_slice,
                            sfT,
                            e_t[:, j * MM:(j + 1) * MM],
                            start=(i == 0),
                            stop=last,
                            skip_group_check=True,
                        )
                        if last:
                            # residual add + store for this bank, overlapped
                            # with the remaining mm2s
                            o2 = out2[half * C:(half + 1) * C,
                                      bank * MM:(bank + 1) * MM]
                            nc.vector.tensor_add(
                                o2, o_slice,
                                f2f[half * C:(half + 1) * C,
                                    j * MM:(j + 1) * MM],
                            )
                            nc.sync.dma_start(out=yv[:, bank, half, :], in_=o2)

                prev = None
                for i in range(NT):
                    e_t = erow.tile([P, N], BF16, tag="e_t")
                    zparts = small.tile([P, NACH], F32, tag="zparts")
                    lhs1 = f2[0:C, i * P:(i + 1) * P]
                    for a in range(NACH):
                        a_t = apsum.tile([P, ACH], F32, tag="a_t")
                        for h in range(2):
                            col = a * ACH + h * MM
                            nc.tensor.matmul(
                                a_t[:, h * MM:(h + 1) * MM],
                                lhs1,
                                f2[0:C, col:col + MM],
                                start=True,
                                stop=True,
                                skip_group_check=True,
                            )
                        nc.scalar.activation(
                            e_t[:, a * ACH:(a + 1) * ACH],
                            a_t,
                            mybir.ActivationFunctionType.Exp,
                            bias=negD[:, i:i + 1],
                            scale=1.0,
                            accum_out=zparts[:, a:a + 1],
                        )
                    z = small.tile([P, 1], F32, tag="z")
                    nc.vector.tensor_reduce(
                        z, zparts, axis=mybir.AxisListType.X,
                        op=mybir.AluOpType.add,
                    )
                    zinv = small.tile([P, 1], F32, tag="zinv")
                    nc.vector.reciprocal(zinv, z)
                    sfT = small.tile([P, C], BF16, tag="sfT")
                    nc.vector.tensor_scalar_mul(
                        sfT, fT[:, i * C:(i + 1) * C], zinv
                    )
                    if prev is not None:
                        emit_mm2(*prev)
                    prev = (i, e_t, sfT)
                emit_mm2(*prev)

    return nc


_NC_CACHE = {}


def _get_nc(mm_dt_name="float32r", repeats=1):
    key = (mm_dt_name, repeats)
    if key not in _NC_CACHE:
        _NC_CACHE[key] = _split_waits(build(mm_dt_name, repeats))
    return _NC_CACHE[key]


def run(x_full, mm_dt_name="float32r", repeats=1):
    """x_full: (B, C, H, W) fp32 -> (B, C, H, W) fp32, sharded over 8 cores."""
    x_full = np.ascontiguousarray(np.asarray(x_full, dtype=np.float32))
    assert x_full.shape == (B, C, H, W)
    nc = _get_nc(mm_dt_name, repeats)
    in_maps = [{"x": x_full[b].reshape(C, N)} for b in range(B)]
    res = run_bass_kernel_spmd(nc, in_maps, list(range(B)))
    out = np.stack([res.results[b]["y"] for b in range(B)])
    return out.reshape(B, C, H, W)


def kernel(**inputs):
    return run(inputs["x"])
